# revision 15
# baseline (speedup 1.0000x reference)
"""Trainium2 Bass kernel for nn_BaselineModel_74509092651544 (CLRS-style MPNN).

Strategy
--------
Data-parallel over graphs: 32 graphs -> 8 cores x 4 graphs.  The dense
[B,N,N,H] message tensor of the reference is never materialized: only the
~61k unique (graph,src,dst) edge slots survive the masked max, so the
message MLP runs on a padded CSR slot layout (~8.5x less compute).

Per core, everything lives in SBUF feature-major [H=128, cols]:
  * node/edge embeddings via one-hot matmuls (host builds integer one-hots),
  * m1[src]+m2[dst]+edge_fts@We accumulated in PSUM via 3 chained matmuls
    (gather matrices G_src/G_dst are host-built 0/1 matrices, resident in
    SBUF, used as the moving operand),
  * the 2-layer message MLP as f32r matmuls at N=512 (full PE rate),
  * masked max over senders as DVE segmented reduces straight out of PSUM
    (receivers are relabeled per graph by in-degree so equal-K groups pack
    into 512-slot tiles; padding duplicates a real slot so no masking is
    needed),
  * LayerNorm in node-major layout via PE transposes,
  * graph pooling + prediction MLP on-device; output is [OUT, 4] per core.

All float math happens on device.  Host work is integer indexing /
relayout only.  Matmuls use float32r (full-rate fp32, ~1.7e-4 rel err).
"""

import sys
import numpy as np

sys.path.insert(0, "/opt/trn_rl_repo")

B, N, H, L, E, OUT = 32, 128, 128, 3, 65536, 128
M = 8                 # NeuronCores
BL = B // M           # graphs per core
NEG = -1e9
EPS = 1e-5
AV, BV = 128, 16

_CACHE = {}


# --------------------------------------------------------------------------
# Host preprocessing: pure integer / relayout work.
# --------------------------------------------------------------------------

def _prep(inputs):
    x = np.asarray(inputs["x"]).astype(np.int64)            # [B*N, 9]
    ea = np.asarray(inputs["edge_attr"]).astype(np.int64)   # [E, 3]
    ei = np.asarray(inputs["edge_index"]).astype(np.int64)  # [2, E]

    g = ei[0] // N
    s = ei[0] % N
    d = ei[1] % N
    key = (g * N + s) * N + d
    uniq, inv = np.unique(key, return_inverse=True)
    US = uniq.size
    ug = uniq // (N * N)
    us = (uniq // N) % N
    ud = uniq % N

    # bond one-hot counts per unique slot  [US, 48]
    oh48 = np.zeros((US, 48), np.float32)
    for c in range(3):
        np.add.at(oh48, (inv, ea[:, c] + 16 * c), 1.0)

    # unique in-degree per (graph, receiver)
    deg = np.zeros((B, N), np.int64)
    np.add.at(deg, (ug, ud), 1)

    # receiver relabeling: position p holds the p-th highest-degree receiver
    rho = np.argsort(-deg, axis=1, kind="stable")        # [B, N] pos -> orig
    rho_inv = np.argsort(rho, axis=1)                    # orig -> pos
    degS = -np.sort(-deg, axis=1)                        # [B, N] desc
    Kp = np.maximum(degS.max(axis=0), 1)                 # [N]

    # group schedule (shared by all graphs/cores): (p0, R, K)
    groups = []
    p = 0
    while p < N:
        K = int(Kp[p])
        if 16 * K <= 512:
            R = 16
        elif 8 * K <= 512:
            R = 8
        else:
            R = 4
        R = min(R, N - p)
        groups.append((p, R, K))
        p += R

    # bin-pack groups into 512-wide slot tiles (first-fit decreasing)
    sizes = [R * K for (_, R, K) in groups]
    order_g = np.argsort(-np.asarray(sizes), kind="stable")
    tiles_used = []
    place = [None] * len(groups)
    for gi in order_g:
        sz = sizes[gi]
        for t in range(len(tiles_used)):
            if tiles_used[t] + sz <= 512:
                place[gi] = (t, tiles_used[t])
                tiles_used[t] += sz
                break
        else:
            place[gi] = (len(tiles_used), 0)
            tiles_used.append(sz)
    n_tiles = len(tiles_used)
    S_graph = 512 * n_tiles
    S_core = BL * S_graph

    # per-position lookup tables
    col_base_of_pos = np.zeros(N, np.int64)   # first column of the receiver
    K_of_pos = np.zeros(N, np.int64)
    for gi, (p0, R, K) in enumerate(groups):
        t, off = place[gi]
        for r in range(R):
            col_base_of_pos[p0 + r] = t * 512 + off + r * K
            K_of_pos[p0 + r] = K

    # slots ordered by (g, d, s): contiguous per receiver
    order = np.lexsort((us, ud, ug))
    og, od, osl = ug[order], ud[order], order
    osrc = us[order]
    recv_id = og * N + od
    first = np.concatenate([[0], np.flatnonzero(np.diff(recv_id)) + 1])
    k_rank = np.arange(len(og)) - first[np.searchsorted(recv_id[first], recv_id)]

    pos = rho_inv[og, od]
    core_r = og // BL
    col_r = (og % BL) * S_graph + col_base_of_pos[pos] + k_rank

    # padding: receivers with deg < K duplicate their first slot
    fg, fd = og[first], od[first]
    fpos = rho_inv[fg, fd]
    fdeg = deg[fg, fd]
    fK = K_of_pos[fpos]
    padc = (fK - fdeg).astype(np.int64)
    assert (padc >= 0).all()
    rep = np.repeat(np.arange(len(first)), padc)
    # k index within each padded receiver: deg .. K-1
    kpad = np.arange(len(rep)) - np.repeat(
        np.concatenate([[0], np.cumsum(padc)[:-1]]), padc
    ) + np.repeat(fdeg, padc)
    pg = fg[rep]
    core_p = pg // BL
    col_p = (pg % BL) * S_graph + col_base_of_pos[fpos[rep]] + kpad
    slot_p = osl[first][rep]
    src_p = osrc[first][rep]
    pos_p = fpos[rep]

    a_core = np.concatenate([core_r, core_p])
    a_col = np.concatenate([col_r, col_p])
    a_slot = np.concatenate([osl, slot_p])
    a_srcnew = np.concatenate(
        [rho_inv[og, osrc], rho_inv[pg, src_p]]
    )
    a_dstpos = np.concatenate([pos, pos_p])

    flat = a_core * S_core + a_col
    Gsrc = np.zeros((M * S_core, 128), np.float32)
    Gdst = np.zeros((M * S_core, 128), np.float32)
    Gsrc[flat, a_srcnew] = 1.0
    Gdst[flat, a_dstpos] = 1.0
    SOH = np.zeros((M * S_core, 48), np.float32)
    SOH[flat] = oh48[a_slot]
    Gsrc = np.ascontiguousarray(Gsrc.reshape(M, S_core, 128).transpose(0, 2, 1))
    Gdst = np.ascontiguousarray(Gdst.reshape(M, S_core, 128).transpose(0, 2, 1))
    SOH = np.ascontiguousarray(SOH.reshape(M, S_core, 48).transpose(0, 2, 1))

    # atom one-hot per core: [9, 128, BL*N] in relabeled node order
    gg = np.repeat(np.arange(B), N)
    pp = np.tile(np.arange(N), B)
    orig = gg * N + rho[gg, pp]                    # [B*N] column -> orig node
    XOH = np.zeros((M, 9, AV, BL * N), np.float32)
    mcol = np.tile(np.arange(BL * N), M)
    mcore = np.repeat(np.arange(M), BL * N)
    for c in range(9):
        XOH[mcore, c, x[orig, c], mcol] = 1.0

    # empty receivers (deg==0) -> need NEG mask path
    empt = (deg == 0)
    has_empty = bool(empt.any())
    maskrow = np.ones((M, BL * N), np.float32)
    negrow = np.zeros((M, BL * N), np.float32)
    if has_empty:
        eg, en = np.nonzero(empt)
        epos = rho_inv[eg, en]
        maskrow[eg // BL, (eg % BL) * N + epos] = 0.0
        negrow[eg // BL, (eg % BL) * N + epos] = NEG

    struct = dict(
        S_graph=S_graph, S_core=S_core, n_tiles=n_tiles,
        groups=[(p0, R, K, place[gi][0], place[gi][1])
                for gi, (p0, R, K) in enumerate(groups)],
        has_empty=has_empty,
    )
    percore = dict(Gsrc=Gsrc, Gdst=Gdst, SOH=SOH, XOH=XOH,
                   maskrow=maskrow, negrow=negrow)
    return struct, percore


def _weight_arrays(inputs):
    f32 = np.float32
    A = {}
    A["atom_emb"] = np.ascontiguousarray(np.asarray(inputs["atom_emb"], f32))
    A["bond_emb"] = np.ascontiguousarray(
        np.asarray(inputs["bond_emb"], f32).reshape(48, H))
    for nm in ("Wm1", "Wm2", "We", "Wp1", "Wp2", "Wo1", "Wo2", "Wh1", "Wh2"):
        A[nm] = np.ascontiguousarray(np.asarray(inputs[nm], f32))
    # bias columns [128, 26]: per l: 4 pre-terms, 2 o-terms, bp1, bp2; + bh1 bh2
    bc = np.zeros((H, 27), f32)
    bc[:, 26] = EPS
    for l in range(L):
        bc[:, 4 * l + 0] = np.asarray(inputs["bm1"], f32)[l]
        bc[:, 4 * l + 1] = np.asarray(inputs["bm2"], f32)[l]
        bc[:, 4 * l + 2] = np.asarray(inputs["be"], f32)[l]
        bc[:, 4 * l + 3] = np.asarray(inputs["bg"], f32)[l]
        bc[:, 12 + 2 * l + 0] = np.asarray(inputs["bo1"], f32)[l]
        bc[:, 12 + 2 * l + 1] = np.asarray(inputs["bo2"], f32)[l]
        bc[:, 18 + l] = np.asarray(inputs["bp1"], f32)[l]
        bc[:, 23 + l] = np.asarray(inputs["bp2"], f32)[l]
    bc[:, 21] = np.asarray(inputs["bh1"], f32)
    bc[:, 22] = np.asarray(inputs["bh2"], f32)[:H]
    A["bias_cols"] = bc
    A["bh2_full"] = np.ascontiguousarray(
        np.asarray(inputs["bh2"], f32).reshape(OUT, 1))
    bp2f = np.zeros((H, 4), f32)
    bp2f[:, :L] = np.asarray(inputs["bp2"], f32).T
    A["bp2f"] = bp2f  # [128, 4] (padded so N=2 slices stay in range)
    A["lnr"] = np.concatenate(
        [np.asarray(inputs["ln_s"], f32), np.asarray(inputs["ln_b"], f32)],
        axis=0)  # [6,128]
    A["idn"] = np.eye(128, dtype=f32)
    return A


# --------------------------------------------------------------------------
# Bass program.
# --------------------------------------------------------------------------

def _build_program(struct):
    import concourse.bacc as bacc
    import concourse.mybir as mybir
    import concourse.tile as tile

    F32 = mybir.dt.float32
    F32R = mybir.dt.float32r
    AF = mybir.ActivationFunctionType
    ALU = mybir.AluOpType
    AX = mybir.AxisListType

    S_graph = struct["S_graph"]
    S_core = struct["S_core"]
    n_tiles = struct["n_tiles"]
    groups = struct["groups"]
    has_empty = struct["has_empty"]

    nc = bacc.Bacc("TRN2", target_bir_lowering=False, debug=False)

    # ---- DRAM tensors
    d_gs = nc.dram_tensor("gsrc", [128, S_core], F32R, kind="ExternalInput")
    d_gd = nc.dram_tensor("gdst", [128, S_core], F32R, kind="ExternalInput")
    d_soh = nc.dram_tensor("soh", [48, S_core], F32R, kind="ExternalInput")
    d_xoh = nc.dram_tensor("xoh", [9, AV, BL * N], F32R, kind="ExternalInput")
    d_atom = nc.dram_tensor("atom_emb", [9, AV, H], F32R, kind="ExternalInput")
    d_bond = nc.dram_tensor("bond_emb", [48, H], F32R, kind="ExternalInput")
    d_wm1 = nc.dram_tensor("Wm1", [L, 2 * H, H], F32R, kind="ExternalInput")
    d_wm2 = nc.dram_tensor("Wm2", [L, 2 * H, H], F32R, kind="ExternalInput")
    d_we = nc.dram_tensor("We", [L, H, H], F32R, kind="ExternalInput")
    d_wp1 = nc.dram_tensor("Wp1", [L, H, H], F32R, kind="ExternalInput")
    d_wp2 = nc.dram_tensor("Wp2", [L, H, H], F32R, kind="ExternalInput")
    d_wo1 = nc.dram_tensor("Wo1", [L, 2 * H, H], F32R, kind="ExternalInput")
    d_wo2 = nc.dram_tensor("Wo2", [L, H, H], F32R, kind="ExternalInput")
    d_wh1 = nc.dram_tensor("Wh1", [H, H], F32R, kind="ExternalInput")
    d_wh2 = nc.dram_tensor("Wh2", [H, OUT], F32R, kind="ExternalInput")
    d_bc = nc.dram_tensor("bias_cols", [H, 27], F32, kind="ExternalInput")
    d_bh2 = nc.dram_tensor("bh2_full", [OUT, 1], F32, kind="ExternalInput")
    d_bp2f = nc.dram_tensor("bp2f", [H, 4], F32R, kind="ExternalInput")
    d_lnr = nc.dram_tensor("lnr", [6, H], F32, kind="ExternalInput")
    d_idn = nc.dram_tensor("idn", [128, 128], F32R, kind="ExternalInput")
    d_mask = nc.dram_tensor("maskrow", [1, BL * N], F32, kind="ExternalInput")
    d_neg = nc.dram_tensor("negrow", [1, BL * N], F32, kind="ExternalInput")
    d_out = nc.dram_tensor("out", [OUT, BL], F32, kind="ExternalOutput")

    with tile.TileContext(nc) as tc:
        _emit(tc, nc, locals(), struct, mybir, F32, F32R, AF, ALU, AX)
    nc.compile()
    return nc


def _emit(tc, nc, d, struct, mybir, F32, F32R, AF, ALU, AX):
    import contextlib
    ctx = contextlib.ExitStack()
    S_graph = struct["S_graph"]
    S_core = struct["S_core"]
    n_tiles = struct["n_tiles"]
    groups = struct["groups"]
    has_empty = struct["has_empty"]

    pG = ctx.enter_context(tc.tile_pool(name="pG", bufs=1))
    pW = ctx.enter_context(tc.tile_pool(name="pW", bufs=1))
    pAct = ctx.enter_context(tc.tile_pool(name="pAct", bufs=3))
    pNM = ctx.enter_context(tc.tile_pool(name="pNM", bufs=1))
    pMB = ctx.enter_context(tc.tile_pool(name="pMB", bufs=2))
    pLN = ctx.enter_context(tc.tile_pool(name="pLN", bufs=2))
    pIn = ctx.enter_context(tc.tile_pool(name="pIn", bufs=2))
    ps_pre = ctx.enter_context(tc.tile_pool(name="ps_pre", bufs=2, space="PSUM"))
    ps_p1 = ctx.enter_context(tc.tile_pool(name="ps_p1", bufs=2, space="PSUM"))
    ps_p2 = ctx.enter_context(tc.tile_pool(name="ps_p2", bufs=2, space="PSUM"))
    ps_misc = ctx.enter_context(tc.tile_pool(name="ps_misc", bufs=2, space="PSUM"))

    def mps(name, dt=F32):
        return ps_misc.tile([128, 512], dt, name=name, tag="mps")

    # ---- resident loads
    gs_sb = pG.tile([128, S_core], F32R, name="gs_sb")
    gd_sb = pG.tile([128, S_core], F32R, name="gd_sb")
    sf_sb = pG.tile([128, S_core], F32R, name="sf_sb")
    for gg in range(BL):
        sl = slice(gg * S_graph, (gg + 1) * S_graph)
        nc.sync.dma_start(gs_sb[:, sl], d["d_gs"].ap()[:, sl])
        nc.sync.dma_start(gd_sb[:, sl], d["d_gd"].ap()[:, sl])

    atom_sb = pW.tile([128, 9 * H], F32R, name="atom_sb")
    for c in range(9):
        nc.sync.dma_start(atom_sb[:, c * H:(c + 1) * H], d["d_atom"].ap()[c])
    bond_sb = pW.tile([48, H], F32R, name="bond_sb")
    nc.sync.dma_start(bond_sb[:], d["d_bond"].ap())
    idn_sb = pW.tile([128, 128], F32R, name="idn_sb")
    nc.sync.dma_start(idn_sb[:], d["d_idn"].ap())
    idn32_sb = pW.tile([128, 128], F32, name="idn32_sb")
    nc.sync.dma_start(idn32_sb[:], d["d_idn"].ap().bitcast(F32))
    bc_sb = pW.tile([H, 27], F32, name="bc_sb")
    nc.sync.dma_start(bc_sb[:], d["d_bc"].ap())
    bh2_sb = pW.tile([OUT, 1], F32, name="bh2_sb")
    nc.sync.dma_start(bh2_sb[:], d["d_bh2"].ap())
    bp2f_sb = pW.tile([H, 4], F32R, name="bp2f_sb")
    nc.sync.dma_start(bp2f_sb[:], d["d_bp2f"].ap())
    lnr_sb = []
    for r in range(6):
        t = pW.tile([1, H], F32, name=f"lnr{r}", tag=f"lnr{r}")
        nc.sync.dma_start(t[:], d["d_lnr"].ap()[r:r + 1, :])
        lnr_sb.append(t)
    W = {}
    for nm, dram, nchunk in (
        ("Wm1", "d_wm1", 2), ("Wm2", "d_wm2", 2), ("Wo1", "d_wo1", 2),
        ("We", "d_we", 1), ("Wp1", "d_wp1", 1), ("Wp2", "d_wp2", 1),
        ("Wo2", "d_wo2", 1),
    ):
        for l in range(L):
            for ch in range(nchunk):
                t = pW.tile([128, H], F32R, name=f"{nm}_{l}_{ch}",
                            tag=f"{nm}_{l}_{ch}")
                nc.sync.dma_start(
                    t[:], d[dram].ap()[l, ch * 128:(ch + 1) * 128, :])
                W[(nm, l, ch)] = t
    wh1_sb = pW.tile([H, H], F32R, name="wh1_sb")
    nc.sync.dma_start(wh1_sb[:], d["d_wh1"].ap())
    wh2_sb = pW.tile([H, OUT], F32R, name="wh2_sb")
    nc.sync.dma_start(wh2_sb[:], d["d_wh2"].ap())

    if has_empty:
        mrow_sb = pW.tile([1, BL * N], F32, name="mrow_sb")
        nc.sync.dma_start(mrow_sb[:], d["d_mask"].ap())
        nrow_sb = pW.tile([1, BL * N], F32, name="nrow_sb")
        nc.sync.dma_start(nrow_sb[:], d["d_neg"].ap())
        mask_bc = pW.tile([128, BL * N], F32, name="mask_bc")
        nc.gpsimd.partition_broadcast(mask_bc[:], mrow_sb[:])
        neg_bc = pW.tile([128, BL * N], F32, name="neg_bc")
        nc.gpsimd.partition_broadcast(neg_bc[:], nrow_sb[:])

    # ---- node features (feature-major) + zero hidden
    nf_ps = mps("nf_ps")
    for c in range(9):
        xoh_sb = pIn.tile([AV, BL * N], F32R, name="xoh_sb", tag="xoh")
        nc.sync.dma_start(xoh_sb[:], d["d_xoh"].ap()[c])
        nc.tensor.matmul(nf_ps[:], atom_sb[:, c * H:(c + 1) * H], xoh_sb[:],
                         start=(c == 0), stop=(c == 8))
    nf = pNM.tile([128, BL * N], F32R, name="nf")
    nc.scalar.activation(nf[:], nf_ps[:], AF.Copy)
    hid0 = pNM.tile([128, BL * N], F32R, name="hid0", tag="hid0")
    nc.scalar.mul(hid0[:], nf[:].bitcast(F32), 0.0)

    # ---- slot features
    for gg in range(BL):
        for t in range(n_tiles):
            c0 = gg * S_graph + t * 512
            soh_sb = pIn.tile([48, 512], F32R, name="soh_sb", tag="soh")
            nc.sync.dma_start(soh_sb[:], d["d_soh"].ap()[:, c0:c0 + 512])
            sf_ps = mps("sf_ps")
            nc.tensor.matmul(sf_ps[:], bond_sb[:], soh_sb[:],
                             start=True, stop=True)
            nc.scalar.activation(sf_sb[:, c0:c0 + 512], sf_ps[:], AF.Copy)

    # bias prework (all layers at once)
    bias_pre = pW.tile([128, L], F32, name="bias_pre")
    nc.vector.tensor_reduce(
        bias_pre[:], bc_sb[:, 0:4 * L].rearrange("p (l f) -> p l f", l=L),
        axis=AX.X, op=ALU.add)
    bo12 = pW.tile([128, L], F32, name="bo12")
    nc.vector.tensor_reduce(
        bo12[:], bc_sb[:, 12:12 + 2 * L].rearrange("p (l f) -> p l f", l=L),
        axis=AX.X, op=ALU.add)

    hid_prev = hid0
    for l in range(L):
        # bias_h = Wo2^T bp2 + bo1 + bo2
        bh_ps = mps("bh_ps")
        nc.tensor.matmul(bh_ps[:, 0:2], W[("Wo2", l, 0)][:],
                         bp2f_sb[:, l:l + 2], start=True, stop=True)
        bh_tmp = pMB.tile([128, 1], F32, name="bh_tmp", tag="bh_tmp")
        nc.scalar.activation(bh_tmp[:], bh_ps[:, 0:1], AF.Copy)
        bias_h = pMB.tile([128, 1], F32, name="bias_h", tag=f"bias_h{l}",
                          bufs=1)
        nc.vector.tensor_tensor(bias_h[:], bh_tmp[:], bo12[:, l:l + 1],
                                op=ALU.add)

        # m1/m2 node-major per graph
        m1_nm, m2_nm = [], []
        for gg in range(BL):
            gsl = slice(gg * N, (gg + 1) * N)
            for which, wnm, dst in (("m1", "Wm1", m1_nm), ("m2", "Wm2", m2_nm)):
                ps_m = mps(f"ps_m_{which}")
                nc.tensor.matmul(ps_m[:, 0:H], nf[:, gsl],
                                 W[(wnm, l, 0)][:], start=True, stop=False)
                nc.tensor.matmul(ps_m[:, 0:H], hid_prev[:, gsl],
                                 W[(wnm, l, 1)][:], start=False, stop=True)
                mt = pMB.tile([128, H], F32R, name=f"{which}_nm{gg}",
                              tag=f"{which}_nm{gg}")
                nc.scalar.activation(mt[:], ps_m[:, 0:H], AF.Copy)
                dst.append(mt)

        msgs_max = pLN.tile([128, BL * N], F32, name="msgs_max",
                            tag="msgs_max")
        for gg in range(BL):
            for t in range(n_tiles):
                c0 = gg * S_graph + t * 512
                pre = ps_pre.tile([128, 512], F32, name="pre")
                nc.tensor.matmul(pre[:], m1_nm[gg][:], gs_sb[:, c0:c0 + 512],
                                 start=True, stop=False)
                nc.tensor.matmul(pre[:], m2_nm[gg][:], gd_sb[:, c0:c0 + 512],
                                 start=False, stop=False)
                nc.tensor.matmul(pre[:], W[("We", l, 0)][:],
                                 sf_sb[:, c0:c0 + 512], start=False, stop=True)
                msgs1 = pAct.tile([128, 512], F32R, name="msgs1", tag="msgs1")
                nc.scalar.activation(msgs1[:], pre[:], AF.Relu,
                                     bias=bias_pre[:, l:l + 1])
                p1 = ps_p1.tile([128, 512], F32, name="p1")
                nc.tensor.matmul(p1[:], W[("Wp1", l, 0)][:], msgs1[:],
                                 start=True, stop=True)
                msgs2 = pAct.tile([128, 512], F32R, name="msgs2", tag="msgs2")
                nc.scalar.activation(msgs2[:], p1[:], AF.Relu,
                                     bias=bc_sb[:, 18 + l:19 + l])
                p2 = ps_p2.tile([128, 512], F32, name="p2")
                nc.tensor.matmul(p2[:], W[("Wp2", l, 0)][:], msgs2[:],
                                 start=True, stop=True)
                for (p0, R, K, gt, off) in groups:
                    if gt != t:
                        continue
                    nc.vector.tensor_reduce(
                        msgs_max[:, gg * N + p0: gg * N + p0 + R],
                        p2[:, off:off + R * K].rearrange(
                            "p (r k) -> p r k", r=R),
                        axis=AX.X, op=ALU.max)

        # + bp2 (deferred), optional empty-receiver masking, -> f32r
        msgs_used = pLN.tile([128, BL * N], F32R, name="msgs_used",
                             tag="msgs_used")
        if has_empty:
            mm1 = pLN.tile([128, BL * N], F32, name="mm1", tag="mm1")
            nc.scalar.activation(mm1[:], msgs_max[:], AF.Identity,
                                 bias=bc_sb[:, 23 + l:24 + l])
            nc.vector.tensor_tensor(mm1[:], mm1[:], mask_bc[:], op=ALU.mult)
            nc.vector.tensor_tensor(mm1[:], mm1[:], neg_bc[:], op=ALU.add)
            nc.scalar.activation(msgs_used[:], mm1[:], AF.Copy)
        else:
            nc.scalar.activation(msgs_used[:], msgs_max[:], AF.Identity,
                                 bias=bc_sb[:, 23 + l:24 + l])

        # h = relu(z @ Wo1 + msgs @ Wo2 + bias_h)   (feature-major)
        h_ps = mps("h_ps")
        nc.tensor.matmul(h_ps[:], W[("Wo1", l, 0)][:], nf[:],
                         start=True, stop=False)
        nc.tensor.matmul(h_ps[:], W[("Wo1", l, 1)][:], hid_prev[:],
                         start=False, stop=False)
        nc.tensor.matmul(h_ps[:], W[("Wo2", l, 0)][:], msgs_used[:],
                         start=False, stop=True)
        h_fm = pLN.tile([128, BL * N], F32R, name="h_fm", tag="h_fm")
        nc.scalar.activation(h_fm[:], h_ps[:], AF.Relu, bias=bias_h[:])

        # LayerNorm in node-major
        sumh = pLN.tile([128, BL], F32, name="sumh", tag="sumh")
        sumsq = pLN.tile([128, BL], F32, name="sumsq", tag="sumsq")
        h_nm = []
        for gg in range(BL):
            gsl = slice(gg * N, (gg + 1) * N)
            tp = mps("tp_ps", F32R)
            nc.tensor.transpose(tp[:, 0:128], h_fm[:, gsl], idn_sb[:])
            hn = pLN.tile([128, 128], F32, name=f"h_nm{gg}", tag=f"h_nm{gg}")
            nc.scalar.activation(hn[:], tp[:, 0:128].bitcast(F32), AF.Copy,
                                 accum_out=sumh[:, gg:gg + 1])
            hsq = pLN.tile([128, 128], F32, name="hsq", tag="hsq")
            nc.scalar.activation(hsq[:], hn[:], AF.Square,
                                 accum_out=sumsq[:, gg:gg + 1])
            h_nm.append(hn)
        negmean = pLN.tile([128, BL], F32, name="negmean", tag="negmean")
        nc.vector.tensor_scalar(negmean[:], sumh[:], -1.0 / H, None,
                                op0=ALU.mult)
        var = pLN.tile([128, BL], F32, name="var", tag="var")
        nc.vector.tensor_scalar(var[:], sumsq[:], 1.0 / H, None, op0=ALU.mult)
        msq = pLN.tile([128, BL], F32, name="msq", tag="msq")
        nc.vector.tensor_tensor(msq[:], negmean[:], negmean[:], op=ALU.mult)
        nc.vector.tensor_tensor(var[:], var[:], msq[:], op=ALU.subtract)
        std = pLN.tile([128, BL], F32, name="std", tag="std")
        nc.scalar.activation(std[:], var[:], AF.Sqrt, bias=bc_sb[:, 26:27])
        rstd = pLN.tile([128, BL], F32, name="rstd", tag="rstd")
        nc.vector.reciprocal(rstd[:], std[:])

        lns_bc = pLN.tile([128, 128], F32, name="lns_bc", tag="lns_bc")
        nc.gpsimd.partition_broadcast(lns_bc[:], lnr_sb[l][:])
        lnb_bc = pLN.tile([128, 128], F32, name="lnb_bc", tag="lnb_bc")
        nc.gpsimd.partition_broadcast(lnb_bc[:], lnr_sb[3 + l][:])

        hid_new = pNM.tile([128, BL * N], F32R, name=f"hid{l + 1}",
                           tag=f"hid{(l + 1) % 2}")
        for gg in range(BL):
            gsl = slice(gg * N, (gg + 1) * N)
            hnorm = pLN.tile([128, 128], F32, name="hnorm", tag="hnorm")
            nc.vector.tensor_scalar(hnorm[:], h_nm[gg][:],
                                    negmean[:, gg:gg + 1], rstd[:, gg:gg + 1],
                                    op0=ALU.add, op1=ALU.mult)
            nc.vector.tensor_tensor(hnorm[:], hnorm[:], lns_bc[:], op=ALU.mult)
            nc.vector.tensor_tensor(hnorm[:], hnorm[:], lnb_bc[:], op=ALU.add)
            tp2 = mps("tp2_ps")
            nc.tensor.transpose(tp2[:, 0:128], hnorm[:], idn32_sb[:])
            nc.scalar.activation(hid_new[:, gsl], tp2[:, 0:128], AF.Copy)
        hid_prev = hid_new

    # ---- pooling + prediction MLP
    ge_sum = pLN.tile([128, BL], F32, name="ge_sum", tag="ge_sum")
    nc.vector.tensor_reduce(
        ge_sum[:], hid_prev[:].bitcast(F32).rearrange("p (g n) -> p g n", g=BL),
        axis=AX.X, op=ALU.add)
    ge = pLN.tile([128, BL], F32R, name="ge", tag="ge")
    nc.scalar.activation(ge[:], ge_sum[:], AF.Copy, scale=1.0 / N)
    o1 = mps("o1_ps")
    nc.tensor.matmul(o1[:, 0:BL], wh1_sb[:], ge[:], start=True, stop=True)
    t1 = pLN.tile([128, BL], F32R, name="t1", tag="t1")
    nc.scalar.activation(t1[:], o1[:, 0:BL], AF.Relu,
                         bias=bc_sb[:, 21:22])
    o2 = mps("o2_ps")
    nc.tensor.matmul(o2[:, 0:BL], wh2_sb[:], t1[:], start=True, stop=True)
    out_sb = pLN.tile([OUT, BL], F32, name="out_sb", tag="out_sb")
    nc.scalar.activation(out_sb[:], o2[:, 0:BL], AF.Identity,
                         bias=bh2_sb[:])
    nc.sync.dma_start(d["d_out"].ap(), out_sb[:])
    ctx.close()


# --------------------------------------------------------------------------
# Entry point.
# --------------------------------------------------------------------------

def build(inputs):
    struct, percore = _prep(inputs)
    A = _weight_arrays(inputs)
    key = (struct["S_graph"], struct["n_tiles"],
           tuple(struct["groups"]), struct["has_empty"])
    if key not in _CACHE:
        _CACHE[key] = _build_program(struct)
    nc = _CACHE[key]

    in_maps = []
    for c in range(M):
        im = dict(
            gsrc=percore["Gsrc"][c], gdst=percore["Gdst"][c],
            soh=percore["SOH"][c], xoh=percore["XOH"][c],
            maskrow=percore["maskrow"][c:c + 1],
            negrow=percore["negrow"][c:c + 1],
        )
        for k, v in A.items():
            im[k] = v
        in_maps.append(im)
    return nc, in_maps, struct


def kernel(**inputs):
    from concourse import bass_utils
    nc, in_maps, struct = build(inputs)
    res = bass_utils.run_bass_kernel_spmd(nc, in_maps, core_ids=list(range(M)))
    out = np.zeros((B, OUT), np.float32)
    for c in range(M):
        out[c * BL:(c + 1) * BL] = res.results[c]["out"].T
    return out
